# revision 2
# baseline (speedup 1.0000x reference)
"""Multi-head attention on 8 trn2 NeuronCores, head-parallel (2 heads/core).

Math per head h (reference semantics):
  Q = query @ Wq[h] + bq[h];  K = key @ Wk[h] + bk[h];  V = query @ Wv[h] + bv[h]
  P = exp(Q K^T / sqrt(D));  alpha = P / rowsum(P)
  ctx = alpha @ V;  y_h = (ctx @ Wp[h] + bp[h]) @ Wo[h]
  out = sum_h y_h + bo

Device-side formulation:
  Since rows of alpha sum to 1, all linear tails fold into the V projection:
    out = sum_h alpha_h @ (X Wv_h Wp_h Wo_h) + const_bias_row
  Per core: project QT/KT = W^T @ xT per head and V'' = X @ (Wv Wp Wo)
  (host-premultiplied), attention with unnormalized softmax (rowsum via
  ones-matmul), y = sum_{h in core} ctx_h / rowsum_h. ReduceScatter the
  [128, S] partial y across the 8 cores per 512-token block; host
  concatenates shards and adds the constant bias row.
"""

import sys

if "/opt/trn_rl_repo" not in sys.path:
    sys.path.insert(0, "/opt/trn_rl_repo")

import ml_dtypes
import numpy as np

import concourse.mybir as mybir
import concourse.tile as tile
from concourse import bacc
from concourse.bass_utils import run_bass_kernel_spmd

B, S = 4, 2048
IN, D, H = 1024, 128, 16
NCORES = 8
HPC = H // NCORES  # heads per core
NCH = IN // 128  # input chunks
TB = 512  # projection token block
NTB = S // TB
QB = 512  # attention query block
NQB = S // QB
KT = 128  # attention key tile
NKT = S // KT
ESH = D // NCORES  # output shard rows per core

f32 = mybir.dt.float32
bf16 = mybir.dt.bfloat16
AF = mybir.ActivationFunctionType

_cache = {}


def build():
    nc = bacc.Bacc(None, target_bir_lowering=False, num_devices=NCORES)

    qT = nc.dram_tensor("qT", [B, IN, S], bf16, kind="ExternalInput")
    kT = nc.dram_tensor("kT", [B, IN, S], bf16, kind="ExternalInput")
    # prepacked [partition, head, chunk, d] so DMA loads are contiguous
    wq = nc.dram_tensor("wq", [128, HPC, NCH, D], bf16, kind="ExternalInput")
    wk = nc.dram_tensor("wk", [128, HPC, NCH, D], bf16, kind="ExternalInput")
    wv = nc.dram_tensor("wv", [128, HPC, NCH, D], bf16, kind="ExternalInput")
    bqT = nc.dram_tensor("bqT", [D, HPC], f32, kind="ExternalInput")
    bkT = nc.dram_tensor("bkT", [D, HPC], f32, kind="ExternalInput")
    onemb = nc.dram_tensor("onemb", [D, D], bf16, kind="ExternalInput")

    out_y = nc.dram_tensor("out_y", [B, NQB, ESH, QB], f32, kind="ExternalOutput")
    y_bounce = [
        [nc.dram_tensor(f"y_bounce{b}_{q}", [D, QB], f32) for q in range(NQB)]
        for b in range(B)
    ]
    y_shard = [
        [nc.dram_tensor(f"y_shard{b}_{q}", [ESH, QB], f32) for q in range(NQB)]
        for b in range(B)
    ]

    scale = 1.0 / float(np.sqrt(D))

    with tile.TileContext(nc) as tc:
        with (
            tc.tile_pool(name="const", bufs=1) as cpool,
            tc.tile_pool(name="xch", bufs=24) as xch,
            tc.tile_pool(name="qkv", bufs=2) as qkv,
            tc.tile_pool(name="work", bufs=2) as work,
            tc.tile_pool(name="pexpp", bufs=4) as pexpp,
            tc.tile_pool(name="ps", bufs=2, space="PSUM") as ps,
        ):
            # ---- resident constants (per-head DMA splits: first matmul only
            # needs wq[:, 0] + the first x chunk) ----
            wq_sb = cpool.tile([128, HPC, NCH, D], bf16, tag="wq_sb")
            wk_sb = cpool.tile([128, HPC, NCH, D], bf16, tag="wk_sb")
            wv_sb = cpool.tile([128, HPC, NCH, D], bf16, tag="wv_sb")
            for h in range(HPC):
                for sb_t, dram_t in ((wq_sb, wq), (wk_sb, wk), (wv_sb, wv)):
                    nc.sync.dma_start(sb_t[:, h], dram_t[:, h])
            bq_sb = cpool.tile([128, HPC], f32, tag="bq_sb")
            bk_sb = cpool.tile([128, HPC], f32, tag="bk_sb")
            nc.sync.dma_start(bq_sb[:], bqT[:])
            nc.sync.dma_start(bk_sb[:], bkT[:])
            onemb_sb = cpool.tile([D, D], bf16, tag="onemb_sb")
            nc.sync.dma_start(onemb_sb[:], onemb[:])

            QTd, KTd, Vnd = {}, {}, {}

            def proj_batch(b):
                # ---- projections: Q & V'' from qT, K from kT ----
                QT = QTd[b] = [qkv.tile([128, S], bf16, tag=f"QT{h}", name=f"QT{h}") for h in range(HPC)]
                KTs = KTd[b] = [qkv.tile([128, S], bf16, tag=f"KT{h}", name=f"KT{h}") for h in range(HPC)]
                Vn = Vnd[b] = [qkv.tile([128, S], bf16, tag=f"VN{h}", name=f"VN{h}") for h in range(HPC)]

                for tb in range(NTB):
                    sl = slice(tb * TB, (tb + 1) * TB)
                    chs = xch.tile([128, NCH, TB], bf16, tag="xch", bufs=3)
                    for c in range(NCH):
                        nc.sync.dma_start(
                            chs[:, c], qT[b, c * 128 : (c + 1) * 128, sl]
                        )
                    pq = ps.tile([128, 2 * TB], f32, tag="pS", name="pq", bufs=2)
                    for h in range(HPC):
                        for c in range(NCH):
                            nc.tensor.matmul(
                                pq[:, h * TB : (h + 1) * TB],
                                wq_sb[:, h, c, :], chs[:, c, :],
                                start=(c == 0), stop=(c == NCH - 1),
                            )
                    for h in range(HPC):
                        with nc.allow_low_precision(reason="f32 psum -> bf16"):
                            nc.vector.tensor_scalar_add(
                                QT[h][:, sl], pq[:, h * TB : (h + 1) * TB],
                                bq_sb[:, h : h + 1],
                            )
                    # V'' in natural [tok, d] layout: chunk subtiles as stationary
                    for t in range(TB // 128):
                        pvt = ps.tile([128, 2 * D], f32, tag="pC", name="pvt", bufs=4)
                        for c in range(NCH):
                            nc.tensor.matmul(
                                pvt[:],
                                chs[:, c, t * 128 : (t + 1) * 128],
                                wv_sb[:, :, c, :],
                                start=(c == 0), stop=(c == NCH - 1),
                            )
                        col = tb * TB + t * 128
                        for h in range(HPC):
                            with nc.allow_low_precision(reason="bf16 PV operand"):
                                nc.vector.tensor_copy(
                                    Vn[h][:, col : col + 128],
                                    pvt[:, h * D : (h + 1) * D],
                                )

                for tb in range(NTB):
                    sl = slice(tb * TB, (tb + 1) * TB)
                    chs = xch.tile([128, NCH, TB], bf16, tag="xch", bufs=3)
                    for c in range(NCH):
                        nc.sync.dma_start(
                            chs[:, c], kT[b, c * 128 : (c + 1) * 128, sl]
                        )
                    pk = ps.tile([128, 2 * TB], f32, tag="pS", name="pk", bufs=2)
                    for h in range(HPC):
                        for c in range(NCH):
                            nc.tensor.matmul(
                                pk[:, h * TB : (h + 1) * TB],
                                wk_sb[:, h, c, :], chs[:, c, :],
                                start=(c == 0), stop=(c == NCH - 1),
                            )
                    for h in range(HPC):
                        with nc.allow_low_precision(reason="f32 psum -> bf16"):
                            nc.vector.tensor_scalar_add(
                                KTs[h][:, sl], pk[:, h * TB : (h + 1) * TB],
                                bk_sb[:, h : h + 1],
                            )

            def attn_batch(b):
                QT, KTs, Vn = QTd.pop(b), KTd.pop(b), Vnd.pop(b)
                # ---- attention: qblock pairs share 2-bank psum + one wide exp ----
                for qbp in range(NQB // 2):
                    q0 = qbp * 2 * QB
                    sl0 = slice(q0, q0 + QB)
                    sl1 = slice(q0 + QB, q0 + 2 * QB)
                    # y accumulator over the core's heads
                    ytile = work.tile([128, 2 * QB], f32, tag="ytile", name="ytile")
                    for h in range(HPC):
                        pctx0 = ps.tile([128, QB], f32, tag="pC", name="pctx0", bufs=4)
                        pctx1 = ps.tile([128, QB], f32, tag="pC", name="pctx1", bufs=4)
                        acc_d = work.tile([128, 2 * QB], bf16, tag="acc_d", name="acc_d")
                        st = [True, None]
                        for kt in range(NKT):
                            ps2 = ps.tile([128, 2 * QB], f32, tag="pS", name="ps2", bufs=2)
                            ksl = slice(kt * 128, (kt + 1) * 128)
                            nc.tensor.matmul(
                                ps2[:, :QB], KTs[h][:, ksl], QT[h][:, sl0],
                                start=True, stop=True,
                            )
                            nc.tensor.matmul(
                                ps2[:, QB:], KTs[h][:, ksl], QT[h][:, sl1],
                                start=True, stop=True,
                            )
                            pexp = pexpp.tile([128, 2 * QB], bf16, tag="pexp", bufs=8)
                            nc.scalar.activation(pexp[:], ps2[:], AF.Exp, scale=scale)
                            nc.tensor.matmul(
                                pctx0[:], Vn[h][:, ksl], pexp[:, :QB],
                                start=(kt == 0), stop=(kt == NKT - 1),
                            )
                            nc.tensor.matmul(
                                pctx1[:], Vn[h][:, ksl], pexp[:, QB:],
                                start=(kt == 0), stop=(kt == NKT - 1),
                            )
                            with nc.allow_low_precision(reason="bf16 rowsum acc"):
                                if st[0] and st[1] is None:
                                    st[1] = pexp
                                elif st[0]:
                                    nc.vector.tensor_add(acc_d[:], st[1][:], pexp[:])
                                    st[0] = False
                                else:
                                    nc.vector.tensor_add(acc_d[:], acc_d[:], pexp[:])
                        # rowsum collapse for this head; hides under the other
                        # head's kt loop. pbc uses a pS slot (free at loop end).
                        pbc = ps.tile([128, 2 * QB], f32, tag="pS", name="pbc", bufs=2)
                        nc.tensor.matmul(
                            pbc[:, :QB], onemb_sb[:], acc_d[:, :QB], start=True, stop=True
                        )
                        nc.tensor.matmul(
                            pbc[:, QB:], onemb_sb[:], acc_d[:, QB:], start=True, stop=True
                        )
                        rsbr = work.tile([128, 2 * QB], f32, tag="rsbr", name="rsbr", bufs=2)
                        nc.vector.reciprocal_approx_fast(out=rsbr[:], in_=pbc[:])
                        if h == 0:
                            for half, pc in ((0, pctx0), (1, pctx1)):
                                hs = slice(half * QB, (half + 1) * QB)
                                nc.vector.tensor_mul(ytile[:, hs], pc[:], rsbr[:, hs])
                        else:
                            ctxn = work.tile([128, 2 * QB], f32, tag="ctxn", name="ctxn")
                            for half, pc in ((0, pctx0), (1, pctx1)):
                                hs = slice(half * QB, (half + 1) * QB)
                                nc.vector.tensor_mul(ctxn[:, hs], pc[:], rsbr[:, hs])
                                # finish y per 512-half so its RS ships early
                                nc.vector.tensor_add(
                                    ytile[:, hs], ytile[:, hs], ctxn[:, hs]
                                )
                                qb = qbp * 2 + half
                                nc.gpsimd.dma_start(
                                    y_bounce[b][qb][:], ytile[:, hs]
                                )
                                nc.gpsimd.collective_compute(
                                    "ReduceScatter",
                                    mybir.AluOpType.add,
                                    replica_groups=[list(range(NCORES))],
                                    ins=[y_bounce[b][qb][:].opt()],
                                    outs=[y_shard[b][qb][:].opt()],
                                )
                                nc.sync.dma_start(
                                    out_y[b, qb], y_shard[b][qb][:]
                                )

            for b in range(B):
                proj_batch(b)
                if b > 0:
                    attn_batch(b - 1)
            attn_batch(B - 1)

    nc.compile()
    return nc


def kernel(**inputs):
    query = np.asarray(inputs["query"], np.float32)
    key = np.asarray(inputs["key"], np.float32)
    Wq, bq = np.asarray(inputs["Wq"], np.float32), np.asarray(inputs["bq"], np.float32)
    Wk, bk = np.asarray(inputs["Wk"], np.float32), np.asarray(inputs["bk"], np.float32)
    Wv, bv = np.asarray(inputs["Wv"], np.float32), np.asarray(inputs["bv"], np.float32)
    Wp, bp = np.asarray(inputs["Wp"], np.float32), np.asarray(inputs["bp"], np.float32)
    Wo, bo = np.asarray(inputs["Wo"], np.float32), np.asarray(inputs["bo"], np.float32)

    qT_b16 = np.ascontiguousarray(query.transpose(0, 2, 1)).astype(ml_dtypes.bfloat16)
    kT_b16 = np.ascontiguousarray(key.transpose(0, 2, 1)).astype(ml_dtypes.bfloat16)

    if "nc" not in _cache:
        _cache["nc"] = build()
    nc = _cache["nc"]

    def prepack(w):  # [HPC, IN, D] -> [128, HPC, NCH, D] contiguous bf16
        return np.ascontiguousarray(
            w.reshape(HPC, NCH, 128, D).transpose(2, 0, 1, 3)
        ).astype(ml_dtypes.bfloat16)

    Wo_h = Wo.reshape(H, D, D)  # rows of Wo per head
    # constant bias row: sum_h (bv_h @ Wp_h + bp_h) @ Wo_h + bo  (host-applied)
    bias_total = (
        np.einsum("hd,hde,hef->f", bv.astype(np.float64), Wp.astype(np.float64), Wo_h.astype(np.float64))
        + np.einsum("hd,hdf->f", bp.astype(np.float64), Wo_h.astype(np.float64))
        + bo.astype(np.float64)
    ).astype(np.float32)

    in_maps = []
    for i in range(NCORES):
        hs = slice(i * HPC, (i + 1) * HPC)
        # premultiplied V-path weight: Wv_h @ Wp_h @ Wo_h  [HPC, IN, D]
        wvpp = np.einsum(
            "hid,hde,hef->hif",
            Wv[hs].astype(np.float64),
            Wp[hs].astype(np.float64),
            Wo_h[hs].astype(np.float64),
        ).astype(np.float32)
        in_maps.append(
            {
                "qT": qT_b16,
                "kT": kT_b16,
                "wq": prepack(Wq[hs]),
                "wk": prepack(Wk[hs]),
                "wv": prepack(wvpp),
                "bqT": np.ascontiguousarray(bq[hs].T),
                "bkT": np.ascontiguousarray(bk[hs].T),
                "onemb": np.ones((D, D), ml_dtypes.bfloat16),
            }
        )

    res = run_bass_kernel_spmd(nc, in_maps, core_ids=list(range(NCORES)))
    _cache["last_result"] = res
    # shards: per core [B, NQB, ESH, QB] -> full [B, S, D]
    parts = np.stack([res.results[i]["out_y"] for i in range(NCORES)], axis=2)
    # [B, NQB, NCORES, ESH, QB] -> [B, NQB, QB, NCORES*ESH] -> [B, S, D]
    yfull = parts.reshape(B, NQB, D, QB).transpose(0, 1, 3, 2).reshape(B, S, D)
    return np.ascontiguousarray(yfull + bias_total[None, None, :])
